# revision 27
# baseline (speedup 1.0000x reference)
"""GQA kernel for Trainium2, sharded over 8 NeuronCores.

Sharding: data-parallel over batch (2) x tensor-parallel over kv_heads (4).
Core c = b*4 + h computes the full attention output partial
    Y_bh = softmax(causal((Q_b @ Wq_eff_h) @ (K_b @ Wk_h)^T / sqrt(dk))) @ (V_b @ Wv_h) @ Wo_h
and the host sums the 4 head partials per batch (the "all-reduce after Wo").
The GQA group-sum-before-softmax quirk folds into the weights:
    Wq_eff_h = sum_g Wq[:, (g*KV+h)*dk : ...].

Bandwidth plan (validated in simulation + on hw):
  - V^T streams as a single e3m4 plane (half the bytes of fp16); wv fp16
    stationary (mixed-dtype matmul, verified on hw). Total max rel err
    1.58e-2, under the 2e-2 gate. VT_E3=False falls back to fp16 V.
  - Everything else fp16 with fp32 PSUM (DoubleRow fp8 measured to give
    no speedup on this hw, so 2-plane fp8 tricks are pointless).

Schedule: K^T, Q^T, V^T all stream column-block-major (512 sequence
positions per block, one strided DMA each). Per block m the chain
  kproj(m) -> qproj(m) -> vproj(m) -> scores(m, c<=4m+3) -> PV(m)
  -> Y(m-1) -> y DMA
runs as soon as block m lands, so attention work starts ~20us in, PE
stays dense (keeping the tensor engine at full p-state), and the y
output DMA overlaps the input stream. Score tiles are processed in
pairs so the scalar-engine exp latency hides behind the next matmul.
DMA wire is the binding resource (~32MB at ~350GB/s per core).
"""
import sys
sys.path.insert(0, '/opt/trn_rl_repo')
import math
import numpy as np
import ml_dtypes

import concourse.bass as bass
import concourse.mybir as mybir
import concourse.tile as tile
from concourse import bacc
from concourse import bass_utils
from concourse.masks import make_identity

FP32 = mybir.dt.float32
FP16 = mybir.dt.float16
E3 = mybir.dt.float8e3
NE3 = ml_dtypes.float8_e3m4

B, L, D = 2, 2048, 2048
Q_HEADS, KV_HEADS, DK, DV = 16, 4, 128, 128
GROUPS = Q_HEADS // KV_HEADS
P = 128
CH = 512                 # sequence block width
NB = L // CH             # 4 blocks
NDC = D // P             # 16 contraction tiles
SCALE_EXP = 1.0 / math.sqrt(DK)
EBIAS = -8.0 * math.log(2.0)   # exp output scaled by 2^-8; cancels in softmax
VT_E3 = True             # V^T as e3m4 single plane (False -> fp16)
VT_DT, VT_NP = (E3, NE3) if VT_E3 else (FP16, np.float16)


def _build():
    nc = bacc.Bacc(trn_type="TRN2")
    kt_d = nc.dram_tensor("kt", (NB, D, CH), FP16, kind="ExternalInput")
    qt_d = nc.dram_tensor("qt", (NB, D, CH), FP16, kind="ExternalInput")
    vt_d = nc.dram_tensor("vt", (NB, D, CH), VT_DT, kind="ExternalInput")
    wq_d = nc.dram_tensor("wq", (P, NDC, DK), FP16, kind="ExternalInput")
    wk_d = nc.dram_tensor("wk", (P, NDC, DK), FP16, kind="ExternalInput")
    wv_d = nc.dram_tensor("wv", (P, NDC, DV), FP16, kind="ExternalInput")
    wo_d = nc.dram_tensor("wo", (DV, D), FP16, kind="ExternalInput")
    mask_d = nc.dram_tensor("mask", (P, NB * CH), FP16, kind="ExternalInput")
    y_d = nc.dram_tensor("y", (L, D), FP16, kind="ExternalOutput")

    with tile.TileContext(nc) as tc:
        with (
            tc.tile_pool(name="const", bufs=1) as const,
            tc.tile_pool(name="wpool", bufs=1) as wpool,
            tc.tile_pool(name="kxs", bufs=2) as kxs,
            tc.tile_pool(name="qxs", bufs=2) as qxs,
            tc.tile_pool(name="vxs", bufs=2) as vxs,
            tc.tile_pool(name="proj", bufs=1) as proj,
            tc.tile_pool(name="etp", bufs=2) as etp,
            tc.tile_pool(name="ev", bufs=3) as ev_pool,
            tc.tile_pool(name="ps", bufs=5, space="PSUM") as ps,
        ):
            ones = const.tile([P, P], FP16)
            nc.vector.memset(ones[:], 1.0)
            ebias = const.tile([P, 1], FP32)
            nc.vector.memset(ebias[:], EBIAS)
            maskt = const.tile([P, NB * CH], FP16)

            kT = proj.tile([P, L], FP16, tag="kT")
            qT = proj.tile([P, L], FP16, tag="qT")
            v_nat = proj.tile([P, L], FP16, tag="v_nat")
            oT = proj.tile([P, L], FP16, tag="oT")
            rinv_all = proj.tile([P, NB * CH], FP32, tag="rinv_all")

            wq = wpool.tile([P, NDC, DK], FP16, tag="wq")
            wk = wpool.tile([P, NDC, DK], FP16, tag="wk")
            wv = wpool.tile([P, NDC, DV], FP16, tag="wv")
            wo_sb = wpool.tile([DV, D], FP16, tag="wo")

            def proj_group(acc, w_sb, x_tile, g, ngroups=4):
                """One quarter of a projection: 4 dc contraction steps."""
                gd = NDC // ngroups
                for dc in range(g * gd, (g + 1) * gd):
                    nc.tensor.matmul(acc[:], w_sb[:, dc, :], x_tile[:, dc, :],
                                     start=(dc == 0), stop=(dc == NDC - 1))

            ets = {}

            def scores(m):
                et_all = etp.tile([P, NDC, CH], FP16, tag="et", name="et")
                ets[m] = et_all
                # score matmuls stream back-to-back on PE; exp (scalar) and
                # mask (vector) chase through the st rotation slots. The
                # rowsum pass runs one block later (rowsum()), by which time
                # the exps have drained.
                for c in range(4 * m + 4):
                    st = ps.tile([P, CH], FP32, tag="ps", name="st")
                    nc.tensor.matmul(st[:], kT[:, c * P:(c + 1) * P],
                                     qT[:, m * CH:(m + 1) * CH],
                                     start=True, stop=True)
                    et = et_all[:, c, :]
                    nc.scalar.activation(et, st[:],
                                         mybir.ActivationFunctionType.Exp,
                                         bias=ebias[:], scale=SCALE_EXP)
                    d = c - 4 * m
                    if d >= 0:   # diagonal tile: zero out k > q
                        nc.vector.tensor_mul(
                            et, et, maskt[:, d * CH:(d + 1) * CH])

            def rowsum(m):
                ncols = 4 * m + 4
                rrep = ps.tile([P, CH], FP32, tag="rrep", bufs=1, name=f"rrep{m}")
                for c in range(ncols):
                    nc.tensor.matmul(rrep[:], ones[:], ets[m][:, c, :],
                                     start=(c == 0), stop=(c == ncols - 1))
                rinv = rinv_all[:, m * CH:(m + 1) * CH]
                nc.vector.reciprocal_approx_fast(rinv, rrep[:])

            def pv(m):
                et_all = ets[m]
                ot = ps.tile([P, CH], FP32, tag="ot", bufs=1, name="ot")
                for c in range(4 * m + 4):
                    nc.tensor.matmul(ot[:], v_nat[:, c * P:(c + 1) * P],
                                     et_all[:, c, :],
                                     start=(c == 0), stop=(c == 4 * m + 3))
                nc.vector.tensor_mul(oT[:, m * CH:(m + 1) * CH], ot[:],
                                     rinv_all[:, m * CH:(m + 1) * CH])

            def y_chunk(m, last=False):
                # last chunk: sync queue (idle by then; HWDGE prep is cheap,
                # avoiding the ~1us/piece SWDGE tail on gpsimd)
                dma_eng = nc.sync if last else nc.gpsimd
                for t in range(CH // P):
                    lq0 = m * CH + t * P
                    yev = ev_pool.tile([P, D], FP16, tag="yev", name="yev")
                    for dch in range(D // CH):
                        yps = ps.tile([P, CH], FP32, tag="ps", name="yps")
                        nc.tensor.matmul(yps[:], oT[:, lq0:lq0 + P],
                                         wo_sb[:, dch * CH:(dch + 1) * CH],
                                         start=True, stop=True)
                        dst = yev[:, dch * CH:(dch + 1) * CH]
                        if dch % 2 == 0:
                            nc.vector.tensor_copy(dst, yps[:])
                        else:
                            nc.scalar.copy(dst, yps[:])
                        # off the sync queue mid-stream so prefetch stays ahead;
                        # per-piece DMA so y bytes hit the wire early
                        dma_eng.dma_start(
                            y_d[lq0:lq0 + P, dch * CH:(dch + 1) * CH], dst)

            nc.scalar.dma_start(wk[:], wk_d[:])
            NG = 4     # sub-DMA groups per block (4 dc tiles each)
            GD = NDC // NG

            for m in range(NB):
                kx = kxs.tile([P, NDC, CH], FP16, tag="kx", name="kx")
                qx = qxs.tile([P, NDC, CH], FP16, tag="qx", name="qx")
                vx = vxs.tile([P, NDC, CH], VT_DT, tag="vx", name="vx")
                srcs = [d[m].rearrange("(g dc p) c -> g p dc c", g=NG, p=P)
                        for d in (kt_d, qt_d, vt_d)]
                for g in range(NG):
                    nc.sync.dma_start(kx[:, g * GD:(g + 1) * GD, :], srcs[0][g])
                if m == 0:
                    nc.scalar.dma_start(wq[:], wq_d[:])
                for g in range(NG):
                    nc.sync.dma_start(qx[:, g * GD:(g + 1) * GD, :], srcs[1][g])
                if m == 0:
                    nc.scalar.dma_start(wv[:], wv_d[:])
                for g in range(NG):
                    nc.sync.dma_start(vx[:, g * GD:(g + 1) * GD, :], srcs[2][g])
                if m == 0:
                    nc.scalar.dma_start(wo_sb[:], wo_d[:])
                    nc.scalar.dma_start(maskt[:], mask_d[:])

                kacc = ps.tile([P, CH], FP32, tag="ps", name=f"kacc{m}")
                qacc = ps.tile([P, CH], FP32, tag="ps", name=f"qacc{m}")
                for g in range(NG):
                    proj_group(kacc, wk, kx, g)
                nc.vector.tensor_copy(kT[:, m * CH:(m + 1) * CH], kacc[:])
                for g in range(NG):
                    proj_group(qacc, wq, qx, g)
                nc.vector.tensor_copy(qT[:, m * CH:(m + 1) * CH], qacc[:])
                # v in natural [lk, dv] layout directly: per 128-row subtile,
                # lhsT = vx dc-slice (stationary), rhs = wv dc-slice (moving).
                # Same cycles as the [dv, lk] projection but no transposes.
                # (each c accumulates alone: PSUM accumulation windows must
                # not interleave within a bank)
                for c in range(4 * m, 4 * m + 4):
                    sub = c * P - m * CH
                    vnacc = ps.tile([P, DV], FP32, tag="ps", name="vnacc")
                    for dc in range(NDC):
                        nc.tensor.matmul(vnacc[:],
                                         vx[:, dc, sub:sub + P],
                                         wv[:, dc, :],
                                         start=(dc == 0), stop=(dc == NDC - 1))
                    nc.scalar.copy(v_nat[:, c * P:(c + 1) * P], vnacc[:])
                # software pipeline: block m-1's rowsum/PV run here, after
                # proj(m) has covered the latency of block m-1's exps
                if m:
                    rowsum(m - 1)
                    pv(m - 1)
                scores(m)
                if m:
                    y_chunk(m - 1)
            rowsum(NB - 1)
            pv(NB - 1)
            y_chunk(NB - 1, last=True)
    nc.compile()
    return nc


_NC = None


def _get_nc():
    global _NC
    if _NC is None:
        _NC = _build()
    return _NC


def _pack_w(w):
    """(D, dk) fp32 -> [P, NDC, dk] fp16: out[p, dc, m] = w[dc*128+p, m]"""
    return np.ascontiguousarray(
        w.reshape(NDC, P, -1).transpose(1, 0, 2)).astype(np.float16)


def _col_blocks(xt, dt):
    """[D, L] -> contiguous (NB, D, CH) in dtype dt."""
    return np.ascontiguousarray(
        xt.reshape(D, NB, CH).transpose(1, 0, 2)).astype(dt)


def _make_in_maps(Q, K, V, Wq, Wk, Wv, Wo):
    f16 = np.float16
    Wq_eff = np.asarray(Wq, np.float32).reshape(D, GROUPS, KV_HEADS, DK).sum(axis=1)
    mask = np.zeros((P, NB * CH), f16)
    for d in range(4):
        p = np.arange(P)[:, None]
        x = np.arange(CH)[None, :]
        mask[:, d * CH:(d + 1) * CH] = (128 * d + p <= x).astype(f16)
    acts = {}
    for b in range(B):
        qt = np.ascontiguousarray(np.asarray(Q[b], np.float32).T)
        kt = np.ascontiguousarray(np.asarray(K[b], np.float32).T)
        vt = np.ascontiguousarray(np.asarray(V[b], np.float32).T)
        acts[b] = {
            "kt": _col_blocks(kt, f16),
            "qt": _col_blocks(qt, f16),
            "vt": _col_blocks(vt, VT_NP),
        }
    Wk32, Wv32 = np.asarray(Wk, np.float32), np.asarray(Wv, np.float32)
    Wo32 = np.asarray(Wo, np.float32)
    in_maps = []
    for c in range(8):
        b, h = divmod(c, KV_HEADS)
        in_maps.append({
            **acts[b],
            "wq": _pack_w(Wq_eff[:, h, :]),
            "wk": _pack_w(Wk32[:, h * DK:(h + 1) * DK]),
            "wv": _pack_w(Wv32[:, h * DV:(h + 1) * DV]),
            "wo": Wo32[h * DV:(h + 1) * DV, :].astype(f16),
            "mask": mask,
        })
    return in_maps


def _gather(results):
    Y = np.zeros((B, L, D), np.float32)
    for c in range(8):
        Y[c // KV_HEADS] += results[c]["y"].astype(np.float32)
    return Y


def kernel(Q, K, V, Wq, Wk, Wv, Wo):
    nc = _get_nc()
    in_maps = _make_in_maps(Q, K, V, Wq, Wk, Wv, Wo)
    res = bass_utils.run_bass_kernel_spmd(nc, in_maps, core_ids=list(range(8)))
    return _gather(res.results)


def _install_ntff_hook():
    """The agent image's antenv lacks axon_hooks; synthesize it so
    trace=True can reach the NTFF profiler in libaxon_pjrt.so."""
    import types
    import antenv
    if hasattr(antenv, "axon_hooks"):
        return
    mod = types.ModuleType("antenv.axon_hooks")
    _h = [None]
    mod.set_axon_ntff_profile_hook = lambda h: _h.__setitem__(0, h)
    mod.get_axon_ntff_profile_hook = lambda: _h[0]
    sys.modules["antenv.axon_hooks"] = mod
    antenv.axon_hooks = mod
    from trn_agent_boot.trn_boot import _ntff_profile_via_ctypes
    mod.set_axon_ntff_profile_hook(_ntff_profile_via_ctypes("/opt/axon/libaxon_pjrt.so"))


def kernel_traced(Q, K, V, Wq, Wk, Wv, Wo):
    """Like kernel() but profiles; returns (output, BassKernelResults)."""
    _install_ntff_hook()
    nc = _get_nc()
    in_maps = _make_in_maps(Q, K, V, Wq, Wk, Wv, Wo)
    res = bass_utils.run_bass_kernel_spmd(nc, in_maps, core_ids=list(range(8)),
                                          trace=True)
    return _gather(res.results), res


# revision 28
# speedup vs baseline: 1.0206x; 1.0206x over previous
"""GQA kernel for Trainium2, sharded over 8 NeuronCores.

Sharding: data-parallel over batch (2) x tensor-parallel over kv_heads (4).
Core c = b*4 + h computes the full attention output partial
    Y_bh = softmax(causal((Q_b @ Wq_eff_h) @ (K_b @ Wk_h)^T / sqrt(dk))) @ (V_b @ Wv_h) @ Wo_h
and the host sums the 4 head partials per batch (the "all-reduce after Wo").
The GQA group-sum-before-softmax quirk folds into the weights:
    Wq_eff_h = sum_g Wq[:, (g*KV+h)*dk : ...].

Bandwidth plan (validated in simulation + on hw):
  - V^T streams as a single e3m4 plane (half the bytes of fp16); wv fp16
    stationary (mixed-dtype matmul, verified on hw). Total max rel err
    1.58e-2, under the 2e-2 gate. VT_E3=False falls back to fp16 V.
  - Everything else fp16 with fp32 PSUM (DoubleRow fp8 measured to give
    no speedup on this hw, so 2-plane fp8 tricks are pointless).

Schedule: K^T, Q^T, V^T all stream column-block-major (512 sequence
positions per block, one strided DMA each). Per block m the chain
  kproj(m) -> qproj(m) -> vproj(m) -> scores(m, c<=4m+3) -> PV(m)
  -> Y(m-1) -> y DMA
runs as soon as block m lands, so attention work starts ~20us in, PE
stays dense (keeping the tensor engine at full p-state), and the y
output DMA overlaps the input stream. Score tiles are processed in
pairs so the scalar-engine exp latency hides behind the next matmul.
DMA wire is the binding resource (~32MB at ~350GB/s per core).
"""
import sys
sys.path.insert(0, '/opt/trn_rl_repo')
import math
import numpy as np
import ml_dtypes

import concourse.bass as bass
import concourse.mybir as mybir
import concourse.tile as tile
from concourse import bacc
from concourse import bass_utils
from concourse.masks import make_identity

FP32 = mybir.dt.float32
FP16 = mybir.dt.float16
E3 = mybir.dt.float8e3
NE3 = ml_dtypes.float8_e3m4

B, L, D = 2, 2048, 2048
Q_HEADS, KV_HEADS, DK, DV = 16, 4, 128, 128
GROUPS = Q_HEADS // KV_HEADS
P = 128
CH = 512                 # sequence block width
NB = L // CH             # 4 blocks
NDC = D // P             # 16 contraction tiles
SCALE_EXP = 1.0 / math.sqrt(DK)
EBIAS = -8.0 * math.log(2.0)   # exp output scaled by 2^-8; cancels in softmax
VT_E3 = True             # V^T as e3m4 single plane (False -> fp16)
VT_DT, VT_NP = (E3, NE3) if VT_E3 else (FP16, np.float16)


def _build():
    nc = bacc.Bacc(trn_type="TRN2")
    kt_d = nc.dram_tensor("kt", (NB, D, CH), FP16, kind="ExternalInput")
    qt_d = nc.dram_tensor("qt", (NB, D, CH), FP16, kind="ExternalInput")
    vt_d = nc.dram_tensor("vt", (NB, D, CH), VT_DT, kind="ExternalInput")
    wq_d = nc.dram_tensor("wq", (P, NDC, DK), FP16, kind="ExternalInput")
    wk_d = nc.dram_tensor("wk", (P, NDC, DK), FP16, kind="ExternalInput")
    wv_d = nc.dram_tensor("wv", (P, NDC, DV), FP16, kind="ExternalInput")
    wo_d = nc.dram_tensor("wo", (DV, D), FP16, kind="ExternalInput")
    mask_d = nc.dram_tensor("mask", (P, NB * CH), FP16, kind="ExternalInput")
    y_d = nc.dram_tensor("y", (L, D), FP16, kind="ExternalOutput")

    with tile.TileContext(nc) as tc:
        with (
            tc.tile_pool(name="const", bufs=1) as const,
            tc.tile_pool(name="wpool", bufs=1) as wpool,
            tc.tile_pool(name="kxs", bufs=2) as kxs,
            tc.tile_pool(name="qxs", bufs=2) as qxs,
            tc.tile_pool(name="vxs", bufs=2) as vxs,
            tc.tile_pool(name="proj", bufs=1) as proj,
            tc.tile_pool(name="etp", bufs=2) as etp,
            tc.tile_pool(name="ev", bufs=4) as ev_pool,
            tc.tile_pool(name="ps", bufs=5, space="PSUM") as ps,
        ):
            ident = const.tile([P, P], FP16)
            make_identity(nc, ident[:])
            ones = const.tile([P, P], FP16)
            nc.vector.memset(ones[:], 1.0)
            ebias = const.tile([P, 1], FP32)
            nc.vector.memset(ebias[:], EBIAS)
            maskt = const.tile([P, NB * CH], FP16)

            kT = proj.tile([P, L], FP16, tag="kT")
            qT = proj.tile([P, L], FP16, tag="qT")
            vT = proj.tile([P, L], FP16, tag="vT")
            v_nat = proj.tile([P, L], FP16, tag="v_nat")
            oT = proj.tile([P, L], FP16, tag="oT")
            rinv_all = proj.tile([P, NB * CH], FP32, tag="rinv_all")

            wq = wpool.tile([P, NDC, DK], FP16, tag="wq")
            wk = wpool.tile([P, NDC, DK], FP16, tag="wk")
            wv = wpool.tile([P, NDC, DV], FP16, tag="wv")
            wo_sb = wpool.tile([DV, D], FP16, tag="wo")

            def proj_group(acc, w_sb, x_tile, g, ngroups=4):
                """One quarter of a projection: 4 dc contraction steps."""
                gd = NDC // ngroups
                for dc in range(g * gd, (g + 1) * gd):
                    nc.tensor.matmul(acc[:], w_sb[:, dc, :], x_tile[:, dc, :],
                                     start=(dc == 0), stop=(dc == NDC - 1))

            ets = {}

            def scores(m):
                et_all = etp.tile([P, NDC, CH], FP16, tag="et", name="et")
                ets[m] = et_all
                # score matmuls stream back-to-back on PE; exp (scalar) and
                # mask (vector) chase through the st rotation slots. The
                # rowsum pass runs one block later (rowsum()), by which time
                # the exps have drained.
                for c in range(4 * m + 4):
                    st = ps.tile([P, CH], FP32, tag="ps", name="st")
                    nc.tensor.matmul(st[:], kT[:, c * P:(c + 1) * P],
                                     qT[:, m * CH:(m + 1) * CH],
                                     start=True, stop=True)
                    et = et_all[:, c, :]
                    nc.scalar.activation(et, st[:],
                                         mybir.ActivationFunctionType.Exp,
                                         bias=ebias[:], scale=SCALE_EXP)
                    d = c - 4 * m
                    if d >= 0:   # diagonal tile: zero out k > q
                        nc.vector.tensor_mul(
                            et, et, maskt[:, d * CH:(d + 1) * CH])

            def rowsum(m):
                ncols = 4 * m + 4
                rrep = ps.tile([P, CH], FP32, tag="rrep", bufs=1, name=f"rrep{m}")
                for c in range(ncols):
                    nc.tensor.matmul(rrep[:], ones[:], ets[m][:, c, :],
                                     start=(c == 0), stop=(c == ncols - 1))
                rinv = rinv_all[:, m * CH:(m + 1) * CH]
                nc.vector.reciprocal_approx_fast(rinv, rrep[:])

            def pv(m):
                et_all = ets[m]
                ot = ps.tile([P, CH], FP32, tag="ot", bufs=1, name="ot")
                for c in range(4 * m + 4):
                    nc.tensor.matmul(ot[:], v_nat[:, c * P:(c + 1) * P],
                                     et_all[:, c, :],
                                     start=(c == 0), stop=(c == 4 * m + 3))
                nc.vector.tensor_mul(oT[:, m * CH:(m + 1) * CH], ot[:],
                                     rinv_all[:, m * CH:(m + 1) * CH])

            def y_chunk(m, last=False):
                # last chunk: sync queue (idle by then; HWDGE prep is cheap,
                # avoiding the ~1us/piece SWDGE tail on gpsimd)
                dma_eng = nc.sync if last else nc.gpsimd
                for t in range(CH // P):
                    lq0 = m * CH + t * P
                    yev = ev_pool.tile([P, D], FP16, tag="yev", name="yev")
                    for dch in range(D // CH):
                        yps = ps.tile([P, CH], FP32, tag="ps", name="yps")
                        nc.tensor.matmul(yps[:], oT[:, lq0:lq0 + P],
                                         wo_sb[:, dch * CH:(dch + 1) * CH],
                                         start=True, stop=True)
                        dst = yev[:, dch * CH:(dch + 1) * CH]
                        if dch % 2 == 0:
                            nc.vector.tensor_copy(dst, yps[:])
                        else:
                            nc.scalar.copy(dst, yps[:])
                        # off the sync queue mid-stream so prefetch stays ahead;
                        # per-piece DMA so y bytes hit the wire early
                        dma_eng.dma_start(
                            y_d[lq0:lq0 + P, dch * CH:(dch + 1) * CH], dst)

            nc.scalar.dma_start(wk[:], wk_d[:])
            NG = 4     # sub-DMA groups per block (4 dc tiles each)
            GD = NDC // NG

            for m in range(NB):
                kx = kxs.tile([P, NDC, CH], FP16, tag="kx", name="kx")
                qx = qxs.tile([P, NDC, CH], FP16, tag="qx", name="qx")
                vx = vxs.tile([P, NDC, CH], VT_DT, tag="vx", name="vx")
                srcs = [d[m].rearrange("(g dc p) c -> g p dc c", g=NG, p=P)
                        for d in (kt_d, qt_d, vt_d)]
                for g in range(NG):
                    nc.sync.dma_start(kx[:, g * GD:(g + 1) * GD, :], srcs[0][g])
                if m == 0:
                    nc.scalar.dma_start(wq[:], wq_d[:])
                for g in range(NG):
                    nc.sync.dma_start(qx[:, g * GD:(g + 1) * GD, :], srcs[1][g])
                if m == 0:
                    nc.scalar.dma_start(wv[:], wv_d[:])
                for g in range(NG):
                    nc.sync.dma_start(vx[:, g * GD:(g + 1) * GD, :], srcs[2][g])
                if m == 0:
                    nc.scalar.dma_start(wo_sb[:], wo_d[:])
                    nc.scalar.dma_start(maskt[:], mask_d[:])

                kacc = ps.tile([P, CH], FP32, tag="ps", name=f"kacc{m}")
                qacc = ps.tile([P, CH], FP32, tag="ps", name=f"qacc{m}")
                for g in range(NG):
                    proj_group(kacc, wk, kx, g)
                nc.vector.tensor_copy(kT[:, m * CH:(m + 1) * CH], kacc[:])
                for g in range(NG):
                    proj_group(qacc, wq, qx, g)
                nc.vector.tensor_copy(qT[:, m * CH:(m + 1) * CH], qacc[:])
                # v projection in [dv, lk] layout (wide moving tiles), then
                # PE-transpose each 128x128 tile into natural [lk, dv] layout
                vacc = ps.tile([P, CH], FP32, tag="ps", name=f"vacc{m}")
                for g in range(NG):
                    proj_group(vacc, wv, vx, g)
                nc.vector.tensor_copy(vT[:, m * CH:(m + 1) * CH], vacc[:])
                for c in range(4 * m, 4 * m + 4):
                    tp = ps.tile([P, P], FP16, tag="ps", name="tp")
                    nc.tensor.transpose(tp[:], vT[:, c * P:(c + 1) * P], ident[:])
                    nc.scalar.copy(v_nat[:, c * P:(c + 1) * P], tp[:])
                # software pipeline: block m-1's rowsum/PV run here, after
                # proj(m) has covered the latency of block m-1's exps
                if m:
                    rowsum(m - 1)
                    pv(m - 1)
                scores(m)
                if m:
                    y_chunk(m - 1)
            rowsum(NB - 1)
            pv(NB - 1)
            y_chunk(NB - 1, last=True)
    nc.compile()
    return nc


_NC = None


def _get_nc():
    global _NC
    if _NC is None:
        _NC = _build()
    return _NC


def _pack_w(w):
    """(D, dk) fp32 -> [P, NDC, dk] fp16: out[p, dc, m] = w[dc*128+p, m]"""
    return np.ascontiguousarray(
        w.reshape(NDC, P, -1).transpose(1, 0, 2)).astype(np.float16)


def _col_blocks(xt, dt):
    """[D, L] -> contiguous (NB, D, CH) in dtype dt."""
    return np.ascontiguousarray(
        xt.reshape(D, NB, CH).transpose(1, 0, 2)).astype(dt)


def _make_in_maps(Q, K, V, Wq, Wk, Wv, Wo):
    f16 = np.float16
    Wq_eff = np.asarray(Wq, np.float32).reshape(D, GROUPS, KV_HEADS, DK).sum(axis=1)
    mask = np.zeros((P, NB * CH), f16)
    for d in range(4):
        p = np.arange(P)[:, None]
        x = np.arange(CH)[None, :]
        mask[:, d * CH:(d + 1) * CH] = (128 * d + p <= x).astype(f16)
    acts = {}
    for b in range(B):
        qt = np.ascontiguousarray(np.asarray(Q[b], np.float32).T)
        kt = np.ascontiguousarray(np.asarray(K[b], np.float32).T)
        vt = np.ascontiguousarray(np.asarray(V[b], np.float32).T)
        acts[b] = {
            "kt": _col_blocks(kt, f16),
            "qt": _col_blocks(qt, f16),
            "vt": _col_blocks(vt, VT_NP),
        }
    Wk32, Wv32 = np.asarray(Wk, np.float32), np.asarray(Wv, np.float32)
    Wo32 = np.asarray(Wo, np.float32)
    in_maps = []
    for c in range(8):
        b, h = divmod(c, KV_HEADS)
        in_maps.append({
            **acts[b],
            "wq": _pack_w(Wq_eff[:, h, :]),
            "wk": _pack_w(Wk32[:, h * DK:(h + 1) * DK]),
            "wv": _pack_w(Wv32[:, h * DV:(h + 1) * DV]),
            "wo": Wo32[h * DV:(h + 1) * DV, :].astype(f16),
            "mask": mask,
        })
    return in_maps


def _gather(results):
    Y = np.zeros((B, L, D), np.float32)
    for c in range(8):
        Y[c // KV_HEADS] += results[c]["y"].astype(np.float32)
    return Y


def kernel(Q, K, V, Wq, Wk, Wv, Wo):
    nc = _get_nc()
    in_maps = _make_in_maps(Q, K, V, Wq, Wk, Wv, Wo)
    res = bass_utils.run_bass_kernel_spmd(nc, in_maps, core_ids=list(range(8)))
    return _gather(res.results)


def _install_ntff_hook():
    """The agent image's antenv lacks axon_hooks; synthesize it so
    trace=True can reach the NTFF profiler in libaxon_pjrt.so."""
    import types
    import antenv
    if hasattr(antenv, "axon_hooks"):
        return
    mod = types.ModuleType("antenv.axon_hooks")
    _h = [None]
    mod.set_axon_ntff_profile_hook = lambda h: _h.__setitem__(0, h)
    mod.get_axon_ntff_profile_hook = lambda: _h[0]
    sys.modules["antenv.axon_hooks"] = mod
    antenv.axon_hooks = mod
    from trn_agent_boot.trn_boot import _ntff_profile_via_ctypes
    mod.set_axon_ntff_profile_hook(_ntff_profile_via_ctypes("/opt/axon/libaxon_pjrt.so"))


def kernel_traced(Q, K, V, Wq, Wk, Wv, Wo):
    """Like kernel() but profiles; returns (output, BassKernelResults)."""
    _install_ntff_hook()
    nc = _get_nc()
    in_maps = _make_in_maps(Q, K, V, Wq, Wk, Wv, Wo)
    res = bass_utils.run_bass_kernel_spmd(nc, in_maps, core_ids=list(range(8)),
                                          trace=True)
    return _gather(res.results), res


# revision 29
# speedup vs baseline: 1.0241x; 1.0035x over previous
"""GQA kernel for Trainium2, sharded over 8 NeuronCores.

Sharding: data-parallel over batch (2) x tensor-parallel over kv_heads (4).
Core c = b*4 + h computes the full attention output partial
    Y_bh = softmax(causal((Q_b @ Wq_eff_h) @ (K_b @ Wk_h)^T / sqrt(dk))) @ (V_b @ Wv_h) @ Wo_h
and the host sums the 4 head partials per batch (the "all-reduce after Wo").
The GQA group-sum-before-softmax quirk folds into the weights:
    Wq_eff_h = sum_g Wq[:, (g*KV+h)*dk : ...].

Bandwidth plan (validated in simulation + on hw):
  - V^T streams as a single e3m4 plane (half the bytes of fp16); wv fp16
    stationary (mixed-dtype matmul, verified on hw). Total max rel err
    1.58e-2, under the 2e-2 gate. VT_E3=False falls back to fp16 V.
  - Everything else fp16 with fp32 PSUM (DoubleRow fp8 measured to give
    no speedup on this hw, so 2-plane fp8 tricks are pointless).

Schedule: K^T, Q^T, V^T all stream column-block-major (512 sequence
positions per block, one strided DMA each). Per block m the chain
  kproj(m) -> qproj(m) -> vproj(m) -> scores(m, c<=4m+3) -> PV(m)
  -> Y(m-1) -> y DMA
runs as soon as block m lands, so attention work starts ~20us in, PE
stays dense (keeping the tensor engine at full p-state), and the y
output DMA overlaps the input stream. Score tiles are processed in
pairs so the scalar-engine exp latency hides behind the next matmul.
DMA wire is the binding resource (~32MB at ~350GB/s per core).
"""
import sys
sys.path.insert(0, '/opt/trn_rl_repo')
import math
import numpy as np
import ml_dtypes

import concourse.bass as bass
import concourse.mybir as mybir
import concourse.tile as tile
from concourse import bacc
from concourse import bass_utils
from concourse.masks import make_identity

FP32 = mybir.dt.float32
FP16 = mybir.dt.float16
E3 = mybir.dt.float8e3
NE3 = ml_dtypes.float8_e3m4

B, L, D = 2, 2048, 2048
Q_HEADS, KV_HEADS, DK, DV = 16, 4, 128, 128
GROUPS = Q_HEADS // KV_HEADS
P = 128
CH = 512                 # sequence block width
NB = L // CH             # 4 blocks
NDC = D // P             # 16 contraction tiles
SCALE_EXP = 1.0 / math.sqrt(DK)
EBIAS = -8.0 * math.log(2.0)   # exp output scaled by 2^-8; cancels in softmax
VT_E3 = True             # V^T as e3m4 single plane (False -> fp16)
VT_DT, VT_NP = (E3, NE3) if VT_E3 else (FP16, np.float16)


def _build():
    nc = bacc.Bacc(trn_type="TRN2")
    kt_d = nc.dram_tensor("kt", (NB, D, CH), FP16, kind="ExternalInput")
    qt_d = nc.dram_tensor("qt", (NB, D, CH), FP16, kind="ExternalInput")
    vt_d = nc.dram_tensor("vt", (NB, D, CH), VT_DT, kind="ExternalInput")
    wq_d = nc.dram_tensor("wq", (P, NDC, DK), FP16, kind="ExternalInput")
    wk_d = nc.dram_tensor("wk", (P, NDC, DK), FP16, kind="ExternalInput")
    wv_d = nc.dram_tensor("wv", (P, NDC, DV), FP16, kind="ExternalInput")
    wo_d = nc.dram_tensor("wo", (DV, D), FP16, kind="ExternalInput")
    mask_d = nc.dram_tensor("mask", (P, NB * CH), FP16, kind="ExternalInput")
    y_d = nc.dram_tensor("y", (L, D), FP16, kind="ExternalOutput")

    with tile.TileContext(nc) as tc:
        with (
            tc.tile_pool(name="const", bufs=1) as const,
            tc.tile_pool(name="wpool", bufs=1) as wpool,
            tc.tile_pool(name="kxs", bufs=2) as kxs,
            tc.tile_pool(name="qxs", bufs=2) as qxs,
            tc.tile_pool(name="vxs", bufs=2) as vxs,
            tc.tile_pool(name="proj", bufs=1) as proj,
            tc.tile_pool(name="etp", bufs=2) as etp,
            tc.tile_pool(name="ev", bufs=4) as ev_pool,
            tc.tile_pool(name="ps", bufs=5, space="PSUM") as ps,
        ):
            ident = const.tile([P, P], FP16)
            make_identity(nc, ident[:])
            ones = const.tile([P, P], FP16)
            nc.vector.memset(ones[:], 1.0)
            ebias = const.tile([P, 1], FP32)
            nc.vector.memset(ebias[:], EBIAS)
            maskt = const.tile([P, NB * CH], FP16)

            kT = proj.tile([P, L], FP16, tag="kT")
            qT = proj.tile([P, L], FP16, tag="qT")
            vT = proj.tile([P, L], FP16, tag="vT")
            v_nat = proj.tile([P, L], FP16, tag="v_nat")
            oT = proj.tile([P, L], FP16, tag="oT")
            rinv_all = proj.tile([P, NB * CH], FP32, tag="rinv_all")

            wq = wpool.tile([P, NDC, DK], FP16, tag="wq")
            wk = wpool.tile([P, NDC, DK], FP16, tag="wk")
            wv = wpool.tile([P, NDC, DV], FP16, tag="wv")
            wo_sb = wpool.tile([DV, D], FP16, tag="wo")

            def proj_group(acc, w_sb, x_tile, g, ngroups=4):
                """One quarter of a projection: 4 dc contraction steps."""
                gd = NDC // ngroups
                for dc in range(g * gd, (g + 1) * gd):
                    nc.tensor.matmul(acc[:], w_sb[:, dc, :], x_tile[:, dc, :],
                                     start=(dc == 0), stop=(dc == NDC - 1))

            ets = {}

            def scores(m):
                et_all = etp.tile([P, NDC, CH], FP16, tag="et", name="et")
                ets[m] = et_all
                # score matmuls stream back-to-back on PE; exp (scalar) and
                # mask (vector) chase through the st rotation slots. The
                # rowsum pass runs one block later (rowsum()), by which time
                # the exps have drained.
                for c in range(4 * m + 4):
                    st = ps.tile([P, CH], FP32, tag="ps", name="st")
                    nc.tensor.matmul(st[:], kT[:, c * P:(c + 1) * P],
                                     qT[:, m * CH:(m + 1) * CH],
                                     start=True, stop=True)
                    et = et_all[:, c, :]
                    nc.scalar.activation(et, st[:],
                                         mybir.ActivationFunctionType.Exp,
                                         bias=ebias[:], scale=SCALE_EXP)
                    d = c - 4 * m
                    if d >= 0:   # diagonal tile: zero out k > q
                        nc.vector.tensor_mul(
                            et, et, maskt[:, d * CH:(d + 1) * CH])

            def rowsum(m):
                ncols = 4 * m + 4
                rrep = ps.tile([P, CH], FP32, tag="rrep", bufs=1, name=f"rrep{m}")
                for c in range(ncols):
                    nc.tensor.matmul(rrep[:], ones[:], ets[m][:, c, :],
                                     start=(c == 0), stop=(c == ncols - 1))
                rinv = rinv_all[:, m * CH:(m + 1) * CH]
                nc.vector.reciprocal_approx_fast(rinv, rrep[:])

            def pv(m):
                et_all = ets[m]
                ot = ps.tile([P, CH], FP32, tag="ot", bufs=1, name="ot")
                for c in range(4 * m + 4):
                    nc.tensor.matmul(ot[:], v_nat[:, c * P:(c + 1) * P],
                                     et_all[:, c, :],
                                     start=(c == 0), stop=(c == 4 * m + 3))
                nc.vector.tensor_mul(oT[:, m * CH:(m + 1) * CH], ot[:],
                                     rinv_all[:, m * CH:(m + 1) * CH])

            def y_chunk(m, last=False):
                # last chunk: sync queue (idle by then; HWDGE prep is cheap,
                # avoiding the ~1us/piece SWDGE tail on gpsimd)
                dma_eng = nc.sync if last else nc.gpsimd
                for t in range(CH // P):
                    lq0 = m * CH + t * P
                    yev = ev_pool.tile([P, D], FP16, tag="yev", name="yev")
                    for dch in range(D // CH):
                        yps = ps.tile([P, CH], FP32, tag="ps", name="yps")
                        nc.tensor.matmul(yps[:], oT[:, lq0:lq0 + P],
                                         wo_sb[:, dch * CH:(dch + 1) * CH],
                                         start=True, stop=True)
                        dst = yev[:, dch * CH:(dch + 1) * CH]
                        if dch % 2 == 0:
                            nc.vector.tensor_copy(dst, yps[:])
                        else:
                            nc.scalar.copy(dst, yps[:])
                        # off the sync queue mid-stream so prefetch stays ahead;
                        # per-piece DMA so y bytes hit the wire early
                        dma_eng.dma_start(
                            y_d[lq0:lq0 + P, dch * CH:(dch + 1) * CH], dst)

            nc.scalar.dma_start(wk[:], wk_d[:])
            NG = 4     # sub-DMA groups per block (4 dc tiles each)
            GD = NDC // NG

            for m in range(NB):
                kx = kxs.tile([P, NDC, CH], FP16, tag="kx", name="kx")
                qx = qxs.tile([P, NDC, CH], FP16, tag="qx", name="qx")
                vx = vxs.tile([P, NDC, CH], VT_DT, tag="vx", name="vx")
                srcs = [d[m].rearrange("(g dc p) c -> g p dc c", g=NG, p=P)
                        for d in (kt_d, qt_d, vt_d)]
                for g in range(NG):
                    nc.sync.dma_start(kx[:, g * GD:(g + 1) * GD, :], srcs[0][g])
                if m == 0:
                    nc.scalar.dma_start(wq[:], wq_d[:])
                for g in range(NG):
                    nc.sync.dma_start(qx[:, g * GD:(g + 1) * GD, :], srcs[1][g])
                if m == 0:
                    nc.scalar.dma_start(wv[:], wv_d[:])
                for g in range(NG):
                    nc.sync.dma_start(vx[:, g * GD:(g + 1) * GD, :], srcs[2][g])
                if m == 0:
                    nc.scalar.dma_start(wo_sb[:], wo_d[:])
                    nc.scalar.dma_start(maskt[:], mask_d[:])

                kacc = ps.tile([P, CH], FP32, tag="ps", name=f"kacc{m}")
                qacc = ps.tile([P, CH], FP32, tag="ps", name=f"qacc{m}")
                for g in range(NG):
                    proj_group(kacc, wk, kx, g)
                nc.vector.tensor_copy(kT[:, m * CH:(m + 1) * CH], kacc[:])
                for g in range(NG):
                    proj_group(qacc, wq, qx, g)
                nc.vector.tensor_copy(qT[:, m * CH:(m + 1) * CH], qacc[:])
                # v projection in [dv, lk] layout (wide moving tiles), then
                # PE-transpose each 128x128 tile into natural [lk, dv] layout
                vacc = ps.tile([P, CH], FP32, tag="ps", name=f"vacc{m}")
                for g in range(NG):
                    proj_group(vacc, wv, vx, g)
                nc.vector.tensor_copy(vT[:, m * CH:(m + 1) * CH], vacc[:])
                for c in range(4 * m, 4 * m + 4):
                    tp = ps.tile([P, P], FP16, tag="ps", name="tp")
                    nc.tensor.transpose(tp[:], vT[:, c * P:(c + 1) * P], ident[:])
                    nc.scalar.copy(v_nat[:, c * P:(c + 1) * P], tp[:])
                # software pipeline: block m-1's rowsum runs after proj(m)
                # covered its exp latency; pv/y of m-1 run after scores(m)
                # so they in turn cover block m's exp drain
                if m:
                    rowsum(m - 1)
                scores(m)
                if m:
                    pv(m - 1)
                    y_chunk(m - 1)
            rowsum(NB - 1)
            pv(NB - 1)
            y_chunk(NB - 1, last=True)
    nc.compile()
    return nc


_NC = None


def _get_nc():
    global _NC
    if _NC is None:
        _NC = _build()
    return _NC


def _pack_w(w):
    """(D, dk) fp32 -> [P, NDC, dk] fp16: out[p, dc, m] = w[dc*128+p, m]"""
    return np.ascontiguousarray(
        w.reshape(NDC, P, -1).transpose(1, 0, 2)).astype(np.float16)


def _col_blocks(xt, dt):
    """[D, L] -> contiguous (NB, D, CH) in dtype dt."""
    return np.ascontiguousarray(
        xt.reshape(D, NB, CH).transpose(1, 0, 2)).astype(dt)


def _make_in_maps(Q, K, V, Wq, Wk, Wv, Wo):
    f16 = np.float16
    Wq_eff = np.asarray(Wq, np.float32).reshape(D, GROUPS, KV_HEADS, DK).sum(axis=1)
    mask = np.zeros((P, NB * CH), f16)
    for d in range(4):
        p = np.arange(P)[:, None]
        x = np.arange(CH)[None, :]
        mask[:, d * CH:(d + 1) * CH] = (128 * d + p <= x).astype(f16)
    acts = {}
    for b in range(B):
        qt = np.ascontiguousarray(np.asarray(Q[b], np.float32).T)
        kt = np.ascontiguousarray(np.asarray(K[b], np.float32).T)
        vt = np.ascontiguousarray(np.asarray(V[b], np.float32).T)
        acts[b] = {
            "kt": _col_blocks(kt, f16),
            "qt": _col_blocks(qt, f16),
            "vt": _col_blocks(vt, VT_NP),
        }
    Wk32, Wv32 = np.asarray(Wk, np.float32), np.asarray(Wv, np.float32)
    Wo32 = np.asarray(Wo, np.float32)
    in_maps = []
    for c in range(8):
        b, h = divmod(c, KV_HEADS)
        in_maps.append({
            **acts[b],
            "wq": _pack_w(Wq_eff[:, h, :]),
            "wk": _pack_w(Wk32[:, h * DK:(h + 1) * DK]),
            "wv": _pack_w(Wv32[:, h * DV:(h + 1) * DV]),
            "wo": Wo32[h * DV:(h + 1) * DV, :].astype(f16),
            "mask": mask,
        })
    return in_maps


def _gather(results):
    Y = np.zeros((B, L, D), np.float32)
    for c in range(8):
        Y[c // KV_HEADS] += results[c]["y"].astype(np.float32)
    return Y


def kernel(Q, K, V, Wq, Wk, Wv, Wo):
    nc = _get_nc()
    in_maps = _make_in_maps(Q, K, V, Wq, Wk, Wv, Wo)
    res = bass_utils.run_bass_kernel_spmd(nc, in_maps, core_ids=list(range(8)))
    return _gather(res.results)


def _install_ntff_hook():
    """The agent image's antenv lacks axon_hooks; synthesize it so
    trace=True can reach the NTFF profiler in libaxon_pjrt.so."""
    import types
    import antenv
    if hasattr(antenv, "axon_hooks"):
        return
    mod = types.ModuleType("antenv.axon_hooks")
    _h = [None]
    mod.set_axon_ntff_profile_hook = lambda h: _h.__setitem__(0, h)
    mod.get_axon_ntff_profile_hook = lambda: _h[0]
    sys.modules["antenv.axon_hooks"] = mod
    antenv.axon_hooks = mod
    from trn_agent_boot.trn_boot import _ntff_profile_via_ctypes
    mod.set_axon_ntff_profile_hook(_ntff_profile_via_ctypes("/opt/axon/libaxon_pjrt.so"))


def kernel_traced(Q, K, V, Wq, Wk, Wv, Wo):
    """Like kernel() but profiles; returns (output, BassKernelResults)."""
    _install_ntff_hook()
    nc = _get_nc()
    in_maps = _make_in_maps(Q, K, V, Wq, Wk, Wv, Wo)
    res = bass_utils.run_bass_kernel_spmd(nc, in_maps, core_ids=list(range(8)),
                                          trace=True)
    return _gather(res.results), res
